# revision 13
# baseline (speedup 1.0000x reference)
"""KNN-memory retrieval kernel for 8x Trainium2 NeuronCores (Bass/Tile).

Problem: sim = x @ queue ([B=4,N=2048,C=1024] @ [C,K=16384]); top-32 of sim
per row; softmax weights; sampled = sum_k w_k * queue[:, idx_k].
Returns (sampled [B,N,C] f32, topk_inds [B,N,32] i32).

Sharding: data-parallel over the 8192 rows (B*N), 1024 rows/core; queue
replicated. Per core:
  phase A: fp32 PE matmul sim tiles (128 rows x 512 cols chunks), PSUM->SBUF,
           per-64-block maxima on DVE, sim spilled to DRAM scratch.
  phase B: top-32 blocks from blockmax (max8/max_index/match_replace), gather
           the 32 winning 64-wide blocks per row back via SWDGE dma_gather
           (2048 candidates/row), exact top-32 + positions, map positions to
           global indices arithmetically, softmax on ACT, gather the 32
           queue.T rows per row (dma_gather) and weighted-sum on ACT+DVE.

fp32 matmul is used for sim (float32r measured at ~1e-3 rel err on HW -> too
lossy for exact top-k index agreement; fp32 PE measured ~6e-7).
"""

import numpy as np

P = 128
DIM = 1024
ROWS = 1024          # rows per core
NRT = ROWS // P      # 8 row-tiles per core
DO = DIM // P        # 8 contraction chunks
K = 16384
KC = 512             # sim column chunk (one PSUM bank, fp32)
NKC = K // KC        # 32
S = 64               # block size for hierarchical top-k
NB = K // S          # 256 blocks per row
TOPK = 32
NCORES = 8
NGROUPS = 2          # row-tile groups; queue is streamed once per group
NEG = -1e30

_CACHE = {}


def _build():
    import os

    import concourse.mybir as mybir
    import concourse.tile as tile
    from concourse import bacc, library_config

    STAGE = int(os.environ.get("KNN_STAGE", "5"))

    f32 = mybir.dt.float32
    u32 = mybir.dt.uint32
    i32 = mybir.dt.int32
    i16 = mybir.dt.int16
    Alu = mybir.AluOpType
    Act = mybir.ActivationFunctionType
    Ax = mybir.AxisListType

    nc = bacc.Bacc("TRN2", target_bir_lowering=False, debug=False,
                   num_devices=NCORES)
    xT = nc.dram_tensor("xT", [DIM, ROWS], f32, kind="ExternalInput").ap()
    queue = nc.dram_tensor("queue", [DIM, K], f32, kind="ExternalInput").ap()
    queueT = nc.dram_tensor("queueT", [K, DIM], f32, kind="ExternalInput").ap()
    sampled = nc.dram_tensor("sampled", [ROWS, DIM], f32,
                             kind="ExternalOutput").ap()
    inds = nc.dram_tensor("inds", [ROWS, TOPK], i32, kind="ExternalOutput").ap()

    from contextlib import ExitStack

    with tile.TileContext(nc) as tc, ExitStack() as ctx:
        const = ctx.enter_context(tc.tile_pool(name="const", bufs=1))
        qpool = ctx.enter_context(tc.tile_pool(name="qch", bufs=2))
        pspool = ctx.enter_context(tc.tile_pool(name="ps", bufs=8,
                                                space="PSUM"))
        scpool = ctx.enter_context(tc.tile_pool(name="sch", bufs=4))
        bmpool = ctx.enter_context(tc.tile_pool(name="bm", bufs=2))
        small = ctx.enter_context(tc.tile_pool(name="small", bufs=4))
        candp = ctx.enter_context(tc.tile_pool(name="cand", bufs=2))
        featp = ctx.enter_context(tc.tile_pool(name="feat", bufs=2))
        partp = ctx.enter_context(tc.tile_pool(name="part", bufs=8))
        dstage = ctx.enter_context(tc.tile_pool(name="dstage", bufs=8,
                                                space="DRAM"))
        dbig = ctx.enter_context(tc.tile_pool(name="dbig", bufs=1,
                                              space="DRAM"))

        nc.gpsimd.load_library(library_config.mlp)

        xT_sb = const.tile([P, DO, ROWS], f32)
        nc.sync.dma_start(xT_sb[:], xT.rearrange("(do p) r -> p do r", p=P))
        iota_u = const.tile([P, 1], u32)
        nc.gpsimd.iota(iota_u[:], pattern=[[0, 1]], base=0,
                       channel_multiplier=NB)
        iota_f = const.tile([P, 1], f32)
        nc.vector.tensor_copy(iota_f[:], iota_u[:])

        simdram = dbig.tile([NRT, P, K], f32)
        q_re = queue.rearrange("(do p) k -> p do k", p=P)

        rts_per_g = NRT // NGROUPS
        for g in range(NGROUPS):
            rts = [rts_per_g * g + i for i in range(rts_per_g)]
            bmax = bmpool.tile([P, rts_per_g, NB], f32)

            # ---- phase A: matmul + blockmax + spill ----
            for kc in range(NKC):
                qch = qpool.tile([P, DO, KC], f32)
                nc.sync.dma_start(qch[:], q_re[:, :, kc * KC:(kc + 1) * KC])
                for i, rt in enumerate(rts):
                    ps = pspool.tile([P, KC], f32)
                    for do in range(DO):
                        nc.tensor.matmul(ps[:],
                                         xT_sb[:, do, rt * P:(rt + 1) * P],
                                         qch[:, do, :],
                                         start=(do == 0), stop=(do == DO - 1))
                    sch = scpool.tile([P, KC], f32)
                    nc.scalar.activation(sch[:], ps[:], Act.Copy)
                    nc.vector.tensor_reduce(
                        bmax[:, i, kc * (KC // S):(kc + 1) * (KC // S)],
                        sch[:].rearrange("p (b t) -> p b t", t=S),
                        axis=Ax.X, op=Alu.max)
                    nc.sync.dma_start(simdram[rt, :, kc * KC:(kc + 1) * KC],
                                      sch[:])

            # ---- phase B per row-tile ----
            for i, rt in enumerate(rts):
                if STAGE < 2:
                    continue
                bm = bmax[:, i, :]
                bidx = small.tile([P, TOPK], u32)
                bv = small.tile([P, TOPK], f32)
                for r in range(4):
                    mx = bv[:, 8 * r:8 * (r + 1)]
                    nc.vector.max(out=mx, in_=bm)
                    nc.vector.max_index(out=bidx[:, 8 * r:8 * (r + 1)],
                                        in_max=mx, in_values=bm)
                    nc.vector.match_replace(out=bm, in_to_replace=mx,
                                            in_values=bm, imm_value=NEG)

                # flat block idx (p*NB + b) in fp32 (exact), to int16
                bidx_f = small.tile([P, TOPK], f32)
                nc.vector.tensor_copy(bidx_f[:], bidx[:])
                fci = small.tile([P, TOPK], f32)
                nc.vector.tensor_scalar(fci[:], bidx_f[:], iota_f[:], None,
                                        op0=Alu.add)
                ci16 = small.tile([P, TOPK], i16)
                nc.vector.tensor_copy(ci16[:], fci[:])
                # stage to DRAM in wrapped layout W[p16, a*8+b] =
                # ci16[16*b + p16, a]: gather j = s*16+p16 must read the
                # gather index for out partition j%128, slot j//128
                cwrap = dstage.tile([16, NB], i16)
                nc.sync.dma_start(
                    cwrap[:].rearrange("p (a b) -> b p a", b=8), ci16[:])
                widxc = small.tile([P, NB], i16)
                nc.sync.dma_start(
                    widxc[:], cwrap[:].unsqueeze(0).broadcast_to([8, 16, NB]))
                if STAGE < 3:
                    continue
                cand = candp.tile([P, TOPK, S], f32)
                simflat = simdram[rt].rearrange("p (b t) -> (p b) t", t=S)
                # 1024-idx chunks: large single SWDGE gathers abort on hw
                for q4 in range(4):
                    nc.gpsimd.dma_gather(
                        cand[:, 8 * q4:8 * (q4 + 1), :], simflat,
                        widxc[:, S * q4:S * (q4 + 1)], 8 * P, 8 * P, S)

                # exact top-32 of the 2048 candidates
                cv = cand[:].rearrange("p b t -> p (b t)")
                vals = small.tile([P, TOPK], f32)
                cpos = small.tile([P, TOPK], u32)
                for r in range(4):
                    mx = vals[:, 8 * r:8 * (r + 1)]
                    nc.vector.max(out=mx, in_=cv)
                    nc.vector.max_index(out=cpos[:, 8 * r:8 * (r + 1)],
                                        in_max=mx, in_values=cv)
                    nc.vector.match_replace(out=cv, in_to_replace=mx,
                                            in_values=cv, imm_value=NEG)

                if STAGE < 4:
                    continue
                # map candidate position -> global column index (fp32 math)
                slot = small.tile([P, TOPK], u32)
                off = small.tile([P, TOPK], u32)
                nc.vector.tensor_scalar(slot[:], cpos[:], 6, None,
                                        op0=Alu.logical_shift_right)
                nc.vector.tensor_scalar(off[:], cpos[:], S - 1, None,
                                        op0=Alu.bitwise_and)
                slot_f = small.tile([P, TOPK], f32)
                nc.vector.tensor_copy(slot_f[:], slot[:])
                off_f = small.tile([P, TOPK], f32)
                nc.vector.tensor_copy(off_f[:], off[:])
                acc = small.tile([P, TOPK], f32)
                nc.vector.memset(acc[:], 0.0)
                for s in range(TOPK):
                    term = small.tile([P, TOPK], f32)
                    nc.vector.tensor_scalar(term[:], slot_f[:], float(s),
                                            bidx_f[:, s:s + 1],
                                            op0=Alu.is_equal, op1=Alu.mult)
                    nc.vector.tensor_tensor(acc[:], acc[:], term[:],
                                            op=Alu.add)
                gidx = small.tile([P, TOPK], f32)
                nc.vector.tensor_scalar(gidx[:], acc[:], float(S), None,
                                        op0=Alu.mult)
                nc.vector.tensor_tensor(gidx[:], gidx[:], off_f[:],
                                        op=Alu.add)
                iout = small.tile([P, TOPK], i32)
                nc.vector.tensor_copy(iout[:], gidx[:])
                nc.sync.dma_start(inds[rt * P:(rt + 1) * P, :], iout[:])

                if STAGE < 5:
                    continue
                # softmax weights over the 32 values
                negm = small.tile([P, 1], f32)
                nc.vector.tensor_scalar_mul(negm[:], vals[:, 0:1], -1.0)
                wexp = small.tile([P, TOPK], f32)
                zacc = small.tile([P, 1], f32)
                nc.scalar.activation(out=wexp[:], in_=vals[:], func=Act.Exp,
                                     bias=negm[:], scale=1.0,
                                     accum_out=zacc[:])
                invz = small.tile([P, 1], f32)
                nc.vector.reciprocal(invz[:], zacc[:])
                w = small.tile([P, TOPK], f32)
                nc.vector.tensor_scalar(w[:], wexp[:], invz[:], None,
                                        op0=Alu.mult)

                # gather the 32 queue.T rows per row and weighted-sum
                gi16 = small.tile([P, TOPK], i16)
                nc.vector.tensor_copy(gi16[:], gidx[:])
                gwrap = dstage.tile([16, NB], i16)
                nc.sync.dma_start(
                    gwrap[:].rearrange("p (a b) -> b p a", b=8), gi16[:])
                parts = []
                for q in range(4):
                    widxf = small.tile([P, S], i16)
                    nc.sync.dma_start(
                        widxf[:],
                        gwrap[:, q * S:(q + 1) * S]
                        .unsqueeze(0).broadcast_to([8, 16, S]))
                    feats = featp.tile([P, 8, DIM], f32)
                    nc.gpsimd.dma_gather(feats[:], queueT, widxf[:],
                                         8 * P, 8 * P, DIM)
                    for k8 in range(8):
                        nc.scalar.activation(
                            out=feats[:, k8, :], in_=feats[:, k8, :],
                            func=Act.Copy,
                            scale=w[:, q * 8 + k8:q * 8 + k8 + 1])
                    pt = partp.tile([P, DIM], f32)
                    nc.vector.tensor_reduce(
                        pt[:], feats[:].rearrange("p k d -> p d k"),
                        axis=Ax.X, op=Alu.add)
                    parts.append(pt)
                nc.vector.tensor_tensor(parts[0][:], parts[0][:],
                                        parts[1][:], op=Alu.add)
                nc.vector.tensor_tensor(parts[2][:], parts[2][:],
                                        parts[3][:], op=Alu.add)
                nc.vector.tensor_tensor(parts[0][:], parts[0][:],
                                        parts[2][:], op=Alu.add)
                nc.sync.dma_start(sampled[rt * P:(rt + 1) * P, :],
                                  parts[0][:])

    nc.compile()
    return nc


def _get_nc():
    if "nc" not in _CACHE:
        _CACHE["nc"] = _build()
    return _CACHE["nc"]


def make_in_maps(x, queue):
    x = np.ascontiguousarray(np.asarray(x, dtype=np.float32))
    queue = np.ascontiguousarray(np.asarray(queue, dtype=np.float32))
    rows = x.reshape(NCORES * ROWS, DIM)
    queueT = np.ascontiguousarray(queue.T)
    in_maps = []
    for c in range(NCORES):
        shard = rows[c * ROWS:(c + 1) * ROWS]
        in_maps.append({
            "xT": np.ascontiguousarray(shard.T),
            "queue": queue,
            "queueT": queueT,
        })
    return in_maps


def assemble(results):
    B, N = 4, 2048
    sampled = np.concatenate([r["sampled"] for r in results], axis=0)
    inds = np.concatenate([r["inds"] for r in results], axis=0)
    return (sampled.reshape(B, N, DIM),
            inds.reshape(B, N, TOPK).astype(np.int32))


def kernel(x, queue):
    from concourse import bass_utils
    nc = _get_nc()
    in_maps = make_in_maps(x, queue)
    res = bass_utils.run_bass_kernel_spmd(nc, in_maps,
                                          core_ids=list(range(NCORES)))
    return assemble(res.results)
